# revision 40
# baseline (speedup 1.0000x reference)
"""MultiHeadEMA (MEGA bidirectional EMA + residual + SiLU) on 8 Trainium2 cores.

Strategy (v2)
-------------
Same math as v1 (banded +-T=64-tap convolution via overlap-save DFT with
the EMA parameters folded into frequency-domain coefficient planes and the
omega residual folded into tap 0), but with a smaller DFT and a paired-
window schedule that cuts TensorE work ~40%:

  F=256, hop C=128, NW=32 windows, processed as 16 supersteps of TWO
  windows packed side by side in the matmul free dimension.  Window c
  needs input tiles {c, c+1} (128 rows each), so the superstep rhs for
  contraction chunk k is the contiguous pair x_all[:, 2p+k : 2p+k+2, :]
  = [128, 1024] -- one matmul covers both windows.

Per superstep (FREE = 2 windows x B x ESH = 1024):
  PE : 4 fwd + 2 inv matmuls, each [128c x 128r x 1024f]  (~0.45 us each)
  ACT: copy XRe->fp16, copy XIm->fp16, silu                (~1.1 us each)
  DVE: m3=XRe*C, m2=XIm*B, m4=XIm*D, YRe=m1-m2, YIm=m3+m4 (~0.7 us each)
  GPS: m1=XRe*A                                           (~2.2 us)

PSUM (8 banks): XRe [128,1024]f32 x1 buf (2) + XIm x2 bufs (4) +
yi [128,1024] x1 buf (2).  XIm is double-buffered because fwd(p+1)'s Im
matmuls would otherwise wait on window p's pointwise reads; XRe is freed
early by the ACT copy.  The inverse for superstep p is emitted one
superstep later (y_sb double-buffered in SBUF) so the pointwise chain
never stalls the PE queue.  Output is stored fp16 (halves the out-DMA).
"""

import math
import numpy as np
from contextlib import ExitStack

import concourse.bass as bass
import concourse.tile as tile
from concourse import bacc, mybir
from concourse.bass_utils import run_bass_kernel_spmd

L, B, E, NDIM = 4096, 4, 1024, 16
N_CORES = 8
ESH = E // N_CORES            # 128 channels per core
F, T, C = 256, 64, 128        # DFT length, one-sided tap support, hop
H = F // 2                    # 128 packed spectral rows per plane
NW = L // C                   # 32 windows
NSS = NW // 2                 # 16 supersteps (2 windows each)
FREE = B * ESH                # 512 free elements (b, chan) per window
FREE2 = 2 * FREE              # 1024: two windows side by side
NXT = (T + L) // 128 + 1      # 33 x tiles: pad[0,64) + x + pad tail

F16 = mybir.dt.float16
F32 = mybir.dt.float32

LAST_RESULTS = None           # BassKernelResults of the most recent run
_CACHE: dict = {}


def _build_nc():
    nc = bacc.Bacc("TRN2", target_bir_lowering=False, debug=False,
                   num_devices=N_CORES)
    # x staged partition-major on the host so every DMA descriptor is a long
    # contiguous run (the (t p) interleave gave 1KB descriptors -> 80 GB/s)
    xs = nc.dram_tensor("xs", [128, NXT, FREE], F16, kind="ExternalInput").ap()
    wf = nc.dram_tensor("wf", [2, 128, 2 * H], F16, kind="ExternalInput").ap()
    vi = nc.dram_tensor("vi", [2, 128, C], F16, kind="ExternalInput").ap()
    # packed coefficient planes: kco[0] = [A | -D], kco[1] = [C | B]
    kco = nc.dram_tensor("kco", [2, 128, 2 * FREE2], F16,
                         kind="ExternalInput").ap()
    # output also partition-major per superstep; host untangles afterwards
    out = nc.dram_tensor("out", [NSS, 128, 2 * FREE], F16,
                         kind="ExternalOutput").ap()

    with ExitStack() as ctx:
        tc = ctx.enter_context(tile.TileContext(nc))
        cpool = ctx.enter_context(tc.tile_pool(name="const", bufs=1))
        ppool = ctx.enter_context(tc.tile_pool(name="pw", bufs=2))
        opool = ctx.enter_context(tc.tile_pool(name="outp", bufs=3))
        pspool = ctx.enter_context(tc.tile_pool(name="ps", bufs=1, space="PSUM"))

        # ---- PE pre-warm FIRST (before any DMA emission) so the PE queue is
        # never head-of-line blocked on a transfer; reads a memset tile.
        wsrc = cpool.tile([128, 2 * H], F16)
        nc.gpsimd.memset(wsrc[:], 0.0)
        warm = pspool.tile([128, FREE2], F32, tag="yi", name="warm")
        for r in range(14):
            nc.tensor.matmul(warm[:, 0:2 * H], wsrc[:, 0:128], wsrc[:],
                             start=(r == 0), stop=(r == 13))

        # ---- constant / input DMA.  x + wf on the sync queue (needed first,
        # smallest-first); the other constants on the idle gpsimd queue.
        x_all = cpool.tile([128, NXT, FREE], F16)
        xr = xs
        nc.sync.dma_start(x_all[:, 0:3, :], xr[:, 0:3, :])
        wf_t = cpool.tile([128, 2, 2 * H], F16)
        nc.sync.dma_start(wf_t[:], wf.transpose([1, 0, 2]))
        vi_t = cpool.tile([128, 2, C], F16)
        nc.scalar.dma_start(vi_t[:], vi.transpose([1, 0, 2]))
        k_t = cpool.tile([128, 2, 2 * FREE2], F16)
        nc.scalar.dma_start(k_t[:], kco.transpose([1, 0, 2]))
        for t0 in range(3, NXT, 6):
            t1 = min(t0 + 6, NXT)
            nc.sync.dma_start(x_all[:, t0:t1, :], xr[:, t0:t1, :])

        def fwd(p, part):
            """forward DFT of superstep p (windows 2p, 2p+1), one spectral
            block: part 0 -> XRe rows, part 1 -> XIm rows."""
            xh = pspool.tile([128, FREE2], F32, tag=f"xh{part}",
                             name=f"xh{part}_{p}", bufs=2 if part == 1 else 1)
            # matmul out free is capped at one PSUM bank (512 fp32) -> two
            # halves (= the two windows), each a contiguous accumulation group
            for h in range(2):
                for k in range(2):
                    nc.tensor.matmul(
                        xh[:, FREE * h:FREE * (h + 1)],
                        wf_t[:, k, 128 * part:128 * (part + 1)],
                        x_all[:, 2 * p + k + h, :],
                        start=(k == 0), stop=(k == 1))
            return xh

        # fwd(0) ahead of the rest of the warm-up: it starts the moment its
        # x tiles land, and the pointwise chain starts ~2us earlier
        xh_cur = [fwd(0, 0), fwd(0, 1)]
        for r in range(6):
            nc.tensor.matmul(warm[:, 0:2 * H], wsrc[:, 0:128], wsrc[:],
                             start=(r == 0), stop=(r == 5))
        xh_next = None
        y_prev = None            # y_sb tiles of superstep p-1
        yi_prev = None

        for p in range(NSS + 1):
            if p < NSS:
                xre, xim = xh_cur
                # ACT: drain PSUM into one wide fp16 tile xx = [XRe | XIm]
                # (frees the XRe bank early for fwd(p+1))
                xx = ppool.tile([128, 2, FREE2], F16, tag="xx", name=f"xx{p}")
                nc.scalar.copy(xx[:, 0, :], xre[:])
                nc.scalar.copy(xx[:, 1, :], xim[:])

            # PE: inverse of superstep p-1 (y_sb double-buffered)
            if y_prev is not None:
                yi = pspool.tile([128, FREE2], F32, tag="yi", name=f"yi{p-1}")
                for h in range(2):
                    for k in range(2):
                        nc.tensor.matmul(yi[:, FREE * h:FREE * (h + 1)],
                                         vi_t[:, k, :],
                                         y_prev[:, k, FREE * h:FREE * (h + 1)],
                                         start=(k == 0), stop=(k == 1))
                yi_prev = yi

            # PE: forward of superstep p+1 (Re first: its bank is freed by
            # the first ACT copy; Im is double-buffered so no wait at all)
            if p + 1 < NSS:
                xh_next = [fwd(p + 1, 0), fwd(p + 1, 1)]

            if p < NSS:
                # pointwise as two wide muls + two subtracts (all on DVE:
                # GpSimd shares SBUF ports with DVE and would slow it down):
                #   mA = [XRe|XIm] * [A|-D] = [m1 | -m4]
                #   mB = [XRe|XIm] * [C| B] = [m3 |  m2]
                #   YRe = m1 - m2 ;  YIm = m3 + m4 = mB0 - mA1
                #   y01 = mA - mB[::-1] = [m1-m2 | -m4-m3] = [YRe | -YIm]
                # (the YIm sign is folded into the inverse matrix on the host)
                mA = ppool.tile([128, 2, FREE2], F16, tag="mA", name=f"mA{p}")
                nc.vector.tensor_mul(mA[:], xx[:], k_t[:, 0, :])
                mB = ppool.tile([128, 2, FREE2], F16, tag="mB", name=f"mB{p}")
                nc.vector.tensor_mul(mB[:], xx[:], k_t[:, 1, :])
                y01 = ppool.tile([128, 2, FREE2], F16, tag="y01", name=f"y{p}")
                nc.vector.tensor_sub(y01[:], mA[:], mB[:, ::-1, :])
                y_cur = y01
            else:
                y_cur = None

            # ACT: silu(p-1) straight out of inverse PSUM, then store fp16
            if yi_prev is not None:
                o_sb = opool.tile([128, 2, FREE], F16, tag="o", name=f"o{p-1}")
                nc.scalar.activation(o_sb[:], yi_prev[:],
                                     mybir.ActivationFunctionType.Silu)
                nc.sync.dma_start(
                    out[p - 1:p].rearrange("s p f -> p (s f)"), o_sb[:])
                yi_prev = None

            y_prev = y_cur
            xh_cur = xh_next
            xh_next = None
    nc.compile()
    return nc


def _host_prep(x, alpha, delta, beta, gamma, omega):
    """Fold the EMA parameters into frequency-domain coefficient planes +
    DFT matrices; shard x/coefs per core."""
    a = 1.0 / (1.0 + np.exp(-alpha.astype(np.float64)))
    d = 1.0 / (1.0 + np.exp(-delta.astype(np.float64)))
    q = (1.0 - a * d)[:, :, 0]                    # (2E, 16)
    w = (a * beta.astype(np.float64))[:, :, 0] * gamma.astype(np.float64)
    w *= math.sqrt(1.0 / NDIM)                    # (2E, 16)
    tau = np.arange(T)
    kern = np.einsum('dn,dnt->dt', w, q[:, :, None] ** tau[None, None, :])
    k1, k2 = kern[:E], kern[E:]                   # (E, T) each
    kc = np.zeros((E, F))
    kc[:, 0:T] = k1
    kc[:, F - T:] = k2[:, ::-1]                   # slot F-i holds k2[i-1]
    kc[:, 0] += omega.astype(np.float64)          # residual == omega on tap 0
    Khat = np.fft.rfft(kc, axis=1)                # (E, H+1)
    KRe, KIm = Khat.real, Khat.imag

    # coefficient planes (H rows x E); row 0 = (DC, Nyquist) special pair
    planes = np.zeros((4, H, E))
    planes[0] = KRe[:, :H].T                      # A (incl. DC at row 0)
    planes[1, 1:] = KIm[:, 1:H].T                 # B
    planes[2, 1:] = KIm[:, 1:H].T                 # C
    planes[3, 1:] = -KRe[:, 1:H].T                # -D (for YIm = mB0 - mA1)
    planes[3, 0] = -KRe[:, H]                     # -Nyquist

    # forward DFT lhsT: [2 kchunk, 128 rows, 2 blocks * 128 cols]
    j = np.arange(F)
    m = np.arange(H)
    ang = 2 * np.pi * np.outer(j, m) / F
    W = np.empty((F, 2, H))
    W[:, 0] = np.cos(ang)                         # Re rows
    W[:, 1] = -np.sin(ang)                        # Im rows
    W[:, 1, 0] = np.cos(np.pi * j)                # row 128: Nyquist (real)
    wf = np.ascontiguousarray(
        W.reshape(2, 128, 2 * H).astype(np.float16))

    # inverse DFT lhsT: [2 kchunk, 128 rows, C cols], positions T..T+C-1
    jj = np.arange(C) + T
    f_lo = np.arange(H)
    V = np.zeros((F, C))
    V[0:H] = np.where(f_lo[:, None] == 0, 1.0, 2.0) \
        * np.cos(2 * np.pi * f_lo[:, None] * jj[None, :] / F) / F
    V[H + 1:] = -2 * np.sin(
        2 * np.pi * np.arange(1, H)[:, None] * jj[None, :] / F) / F
    V[H] = ((-1.0) ** jj) / F                     # Nyquist row
    V[H:] *= -1.0    # device computes -YIm; fold the sign into the inverse
    vi = np.ascontiguousarray(V.reshape(2, 128, C).astype(np.float16))

    xpad = np.zeros((NXT * 128, B, E), np.float16)
    xpad[T:T + L] = x.astype(np.float16)
    # partition-major: [p, t, b, e] so per-partition DMA reads are contiguous
    xpm = xpad.reshape(NXT, 128, B, E).transpose(1, 0, 2, 3)

    in_maps = []
    for core in range(N_CORES):
        sl = slice(core * ESH, (core + 1) * ESH)
        # duplicate each plane for both windows -> [4, 128, FREE2],
        # then pack as kco[0] = [A | -D], kco[1] = [C | B]
        pl = np.broadcast_to(
            planes.reshape(4, H, 1, 1, E)[:, :, :, :, sl],
            (4, H, 2, B, ESH)).reshape(4, 128, FREE2)
        kco = np.stack([
            np.concatenate([pl[0], pl[3]], axis=1),   # [A | -D]
            np.concatenate([pl[2], pl[1]], axis=1),   # [C |  B]
        ])
        in_maps.append({
            "xs": np.ascontiguousarray(
                xpm[:, :, :, sl]).reshape(128, NXT, FREE),
            "wf": wf,
            "vi": vi,
            "kco": np.ascontiguousarray(kco.astype(np.float16)),
        })
    return in_maps


def kernel(x, alpha, delta, beta, gamma, omega):
    global LAST_RESULTS
    if "nc" not in _CACHE:
        _CACHE["nc"] = _build_nc()
    nc = _CACHE["nc"]
    in_maps = _host_prep(x, alpha, delta, beta, gamma, omega)
    res = run_bass_kernel_spmd(nc, in_maps, core_ids=list(range(N_CORES)))
    LAST_RESULTS = res
    # untangle the partition-major store: [ss, p, (w b c)] -> [L, B, ESH]
    outs = []
    for c in range(N_CORES):
        o = res.results[c]["out"].reshape(NSS, 128, 2, B, ESH)
        outs.append(o.transpose(0, 2, 1, 3, 4).reshape(L, B, ESH))
    return np.concatenate(outs, axis=2).astype(np.float32)


# revision 47
# speedup vs baseline: 1.0120x; 1.0120x over previous
"""MultiHeadEMA (MEGA bidirectional EMA + residual + SiLU) on 8 Trainium2 cores.

Strategy (v2)
-------------
Same math as v1 (banded +-T=64-tap convolution via overlap-save DFT with
the EMA parameters folded into frequency-domain coefficient planes and the
omega residual folded into tap 0), but with a smaller DFT and a paired-
window schedule that cuts TensorE work ~40%:

  F=256, hop C=128, NW=32 windows, processed as 16 supersteps of TWO
  windows packed side by side in the matmul free dimension.  Window c
  needs input tiles {c, c+1} (128 rows each), so the superstep rhs for
  contraction chunk k is the contiguous pair x_all[:, 2p+k : 2p+k+2, :]
  = [128, 1024] -- one matmul covers both windows.

Per superstep (FREE = 2 windows x B x ESH = 1024):
  PE : 4 fwd + 2 inv matmuls, each [128c x 128r x 1024f]  (~0.45 us each)
  ACT: copy XRe->fp16, copy XIm->fp16, silu                (~1.1 us each)
  DVE: m3=XRe*C, m2=XIm*B, m4=XIm*D, YRe=m1-m2, YIm=m3+m4 (~0.7 us each)
  GPS: m1=XRe*A                                           (~2.2 us)

PSUM (8 banks): XRe [128,1024]f32 x1 buf (2) + XIm x2 bufs (4) +
yi [128,1024] x1 buf (2).  XIm is double-buffered because fwd(p+1)'s Im
matmuls would otherwise wait on window p's pointwise reads; XRe is freed
early by the ACT copy.  The inverse for superstep p is emitted one
superstep later (y_sb double-buffered in SBUF) so the pointwise chain
never stalls the PE queue.  Output is stored fp16 (halves the out-DMA).
"""

import math
import numpy as np
from contextlib import ExitStack

import concourse.bass as bass
import concourse.tile as tile
from concourse import bacc, mybir
from concourse.bass_utils import run_bass_kernel_spmd

L, B, E, NDIM = 4096, 4, 1024, 16
N_CORES = 8
ESH = E // N_CORES            # 128 channels per core
F, T, C = 256, 64, 128        # DFT length, one-sided tap support, hop
H = F // 2                    # 128 packed spectral rows per plane
NW = L // C                   # 32 windows
NSS = NW // 2                 # 16 supersteps (2 windows each)
FREE = B * ESH                # 512 free elements (b, chan) per window
FREE2 = 2 * FREE              # 1024: two windows side by side
NXT = (T + L) // 128 + 1      # 33 x tiles: pad[0,64) + x + pad tail

F16 = mybir.dt.float16
F32 = mybir.dt.float32

LAST_RESULTS = None           # BassKernelResults of the most recent run
_CACHE: dict = {}


def _build_nc():
    nc = bacc.Bacc("TRN2", target_bir_lowering=False, debug=False,
                   num_devices=N_CORES)
    # x staged partition-major on the host so every DMA descriptor is a long
    # contiguous run (the (t p) interleave gave 1KB descriptors -> 80 GB/s)
    xs = nc.dram_tensor("xs", [128, NXT, FREE], F16, kind="ExternalInput").ap()
    wf = nc.dram_tensor("wf", [2, 128, 2 * H], F16, kind="ExternalInput").ap()
    vi = nc.dram_tensor("vi", [2, 128, C], F16, kind="ExternalInput").ap()
    # packed coefficient planes: kco[0] = [A | -D], kco[1] = [C | B]
    kco = nc.dram_tensor("kco", [2, 128, 2 * FREE2], F16,
                         kind="ExternalInput").ap()
    # output also partition-major per superstep; host untangles afterwards
    out = nc.dram_tensor("out", [NSS, 128, 2 * FREE], F16,
                         kind="ExternalOutput").ap()

    with ExitStack() as ctx:
        tc = ctx.enter_context(tile.TileContext(nc))
        cpool = ctx.enter_context(tc.tile_pool(name="const", bufs=1))
        ppool = ctx.enter_context(tc.tile_pool(name="pw", bufs=3))
        opool = ctx.enter_context(tc.tile_pool(name="outp", bufs=3))
        pspool = ctx.enter_context(tc.tile_pool(name="ps", bufs=1, space="PSUM"))

        # ---- PE pre-warm FIRST (before any DMA emission) so the PE queue is
        # never head-of-line blocked on a transfer; reads a memset tile.
        wsrc = cpool.tile([128, 2 * H], F16)
        nc.gpsimd.memset(wsrc[:], 0.0)
        warm = pspool.tile([128, FREE2], F32, tag="yi", name="warm")
        for r in range(8):
            nc.tensor.matmul(warm[:, 0:2 * H], wsrc[:, 0:128], wsrc[:],
                             start=(r == 0), stop=(r == 7))

        # ---- constant / input DMA.  x + wf on the sync queue (needed first,
        # smallest-first); the other constants on the idle gpsimd queue.
        x_all = cpool.tile([128, NXT, FREE], F16)
        xr = xs
        nc.sync.dma_start(x_all[:, 0:3, :], xr[:, 0:3, :])
        wf_t = cpool.tile([128, 2, 2 * H], F16)
        nc.sync.dma_start(wf_t[:], wf.transpose([1, 0, 2]))
        # vi/kco also on sync (a DMA issue occupies its queue through the
        # transfer -- keeping them off the scalar queue lets the first ACT
        # copies start right after fwd(0) instead of behind a 1MB transfer)
        vi_t = cpool.tile([128, 2, C], F16)
        nc.sync.dma_start(vi_t[:], vi.transpose([1, 0, 2]))
        k_t = cpool.tile([128, 2, 2 * FREE2], F16)
        nc.sync.dma_start(k_t[:], kco.transpose([1, 0, 2]))
        for t0 in range(3, NXT, 6):
            t1 = min(t0 + 6, NXT)
            nc.sync.dma_start(x_all[:, t0:t1, :], xr[:, t0:t1, :])

        def fwd(p, part):
            """forward DFT of superstep p (windows 2p, 2p+1), one spectral
            block: part 0 -> XRe rows, part 1 -> XIm rows."""
            xh = pspool.tile([128, FREE2], F32, tag=f"xh{part}",
                             name=f"xh{part}_{p}", bufs=2 if part == 1 else 1)
            # matmul out free is capped at one PSUM bank (512 fp32) -> two
            # halves (= the two windows), each a contiguous accumulation group
            for h in range(2):
                for k in range(2):
                    nc.tensor.matmul(
                        xh[:, FREE * h:FREE * (h + 1)],
                        wf_t[:, k, 128 * part:128 * (part + 1)],
                        x_all[:, 2 * p + k + h, :],
                        start=(k == 0), stop=(k == 1))
            return xh

        xh_cur = [fwd(0, 0), fwd(0, 1)]
        xh_next = None
        y_prev = None            # y_sb tiles of superstep p-1
        yi_prev = None

        for p in range(NSS + 1):
            if p < NSS:
                xre, xim = xh_cur
                # ACT: drain PSUM into one wide fp16 tile xx = [XRe | XIm]
                # (frees the XRe bank early for fwd(p+1))
                xx = ppool.tile([128, 2, FREE2], F16, tag="xx", name=f"xx{p}")
                nc.scalar.copy(xx[:, 0, :], xre[:])
                nc.scalar.copy(xx[:, 1, :], xim[:])

            # PE: inverse of superstep p-1 (y_sb double-buffered)
            if y_prev is not None:
                yi = pspool.tile([128, FREE2], F32, tag="yi", name=f"yi{p-1}")
                for h in range(2):
                    for k in range(2):
                        nc.tensor.matmul(yi[:, FREE * h:FREE * (h + 1)],
                                         vi_t[:, k, :],
                                         y_prev[:, k, FREE * h:FREE * (h + 1)],
                                         start=(k == 0), stop=(k == 1))
                yi_prev = yi

            # PE: forward of superstep p+1 (Re first: its bank is freed by
            # the first ACT copy; Im is double-buffered so no wait at all)
            if p + 1 < NSS:
                xh_next = [fwd(p + 1, 0), fwd(p + 1, 1)]

            if p < NSS:
                # pointwise as two wide muls + two subtracts (all on DVE:
                # GpSimd shares SBUF ports with DVE and would slow it down):
                #   mA = [XRe|XIm] * [A|-D] = [m1 | -m4]
                #   mB = [XRe|XIm] * [C| B] = [m3 |  m2]
                #   YRe = m1 - m2 ;  YIm = m3 + m4 = mB0 - mA1
                #   y01 = mA - mB[::-1] = [m1-m2 | -m4-m3] = [YRe | -YIm]
                # (the YIm sign is folded into the inverse matrix on the host)
                mA = ppool.tile([128, 2, FREE2], F16, tag="mA", name=f"mA{p}")
                nc.vector.tensor_mul(mA[:], xx[:], k_t[:, 0, :])
                mB = ppool.tile([128, 2, FREE2], F16, tag="mB", name=f"mB{p}")
                nc.vector.tensor_mul(mB[:], xx[:], k_t[:, 1, :])
                y01 = ppool.tile([128, 2, FREE2], F16, tag="y01", name=f"y{p}")
                nc.vector.tensor_sub(y01[:], mA[:], mB[:, ::-1, :])
                y_cur = y01
            else:
                y_cur = None

            # ACT: silu(p-1) straight out of inverse PSUM, then store fp16
            if yi_prev is not None:
                o_sb = opool.tile([128, 2, FREE], F16, tag="o", name=f"o{p-1}")
                nc.scalar.activation(o_sb[:], yi_prev[:],
                                     mybir.ActivationFunctionType.Silu)
                nc.sync.dma_start(
                    out[p - 1:p].rearrange("s p f -> p (s f)"), o_sb[:])
                yi_prev = None

            y_prev = y_cur
            xh_cur = xh_next
            xh_next = None
    nc.compile()
    return nc


def _host_prep(x, alpha, delta, beta, gamma, omega):
    """Fold the EMA parameters into frequency-domain coefficient planes +
    DFT matrices; shard x/coefs per core."""
    a = 1.0 / (1.0 + np.exp(-alpha.astype(np.float64)))
    d = 1.0 / (1.0 + np.exp(-delta.astype(np.float64)))
    q = (1.0 - a * d)[:, :, 0]                    # (2E, 16)
    w = (a * beta.astype(np.float64))[:, :, 0] * gamma.astype(np.float64)
    w *= math.sqrt(1.0 / NDIM)                    # (2E, 16)
    tau = np.arange(T)
    kern = np.einsum('dn,dnt->dt', w, q[:, :, None] ** tau[None, None, :])
    k1, k2 = kern[:E], kern[E:]                   # (E, T) each
    kc = np.zeros((E, F))
    kc[:, 0:T] = k1
    kc[:, F - T:] = k2[:, ::-1]                   # slot F-i holds k2[i-1]
    kc[:, 0] += omega.astype(np.float64)          # residual == omega on tap 0
    Khat = np.fft.rfft(kc, axis=1)                # (E, H+1)
    KRe, KIm = Khat.real, Khat.imag

    # coefficient planes (H rows x E); row 0 = (DC, Nyquist) special pair
    planes = np.zeros((4, H, E))
    planes[0] = KRe[:, :H].T                      # A (incl. DC at row 0)
    planes[1, 1:] = KIm[:, 1:H].T                 # B
    planes[2, 1:] = KIm[:, 1:H].T                 # C
    planes[3, 1:] = -KRe[:, 1:H].T                # -D (for YIm = mB0 - mA1)
    planes[3, 0] = -KRe[:, H]                     # -Nyquist

    # forward DFT lhsT: [2 kchunk, 128 rows, 2 blocks * 128 cols]
    j = np.arange(F)
    m = np.arange(H)
    ang = 2 * np.pi * np.outer(j, m) / F
    W = np.empty((F, 2, H))
    W[:, 0] = np.cos(ang)                         # Re rows
    W[:, 1] = -np.sin(ang)                        # Im rows
    W[:, 1, 0] = np.cos(np.pi * j)                # row 128: Nyquist (real)
    wf = np.ascontiguousarray(
        W.reshape(2, 128, 2 * H).astype(np.float16))

    # inverse DFT lhsT: [2 kchunk, 128 rows, C cols], positions T..T+C-1
    jj = np.arange(C) + T
    f_lo = np.arange(H)
    V = np.zeros((F, C))
    V[0:H] = np.where(f_lo[:, None] == 0, 1.0, 2.0) \
        * np.cos(2 * np.pi * f_lo[:, None] * jj[None, :] / F) / F
    V[H + 1:] = -2 * np.sin(
        2 * np.pi * np.arange(1, H)[:, None] * jj[None, :] / F) / F
    V[H] = ((-1.0) ** jj) / F                     # Nyquist row
    V[H:] *= -1.0    # device computes -YIm; fold the sign into the inverse
    vi = np.ascontiguousarray(V.reshape(2, 128, C).astype(np.float16))

    xpad = np.zeros((NXT * 128, B, E), np.float16)
    xpad[T:T + L] = x.astype(np.float16)
    # partition-major: [p, t, b, e] so per-partition DMA reads are contiguous
    xpm = xpad.reshape(NXT, 128, B, E).transpose(1, 0, 2, 3)

    in_maps = []
    for core in range(N_CORES):
        sl = slice(core * ESH, (core + 1) * ESH)
        # duplicate each plane for both windows -> [4, 128, FREE2],
        # then pack as kco[0] = [A | -D], kco[1] = [C | B]
        pl = np.broadcast_to(
            planes.reshape(4, H, 1, 1, E)[:, :, :, :, sl],
            (4, H, 2, B, ESH)).reshape(4, 128, FREE2)
        kco = np.stack([
            np.concatenate([pl[0], pl[3]], axis=1),   # [A | -D]
            np.concatenate([pl[2], pl[1]], axis=1),   # [C |  B]
        ])
        in_maps.append({
            "xs": np.ascontiguousarray(
                xpm[:, :, :, sl]).reshape(128, NXT, FREE),
            "wf": wf,
            "vi": vi,
            "kco": np.ascontiguousarray(kco.astype(np.float16)),
        })
    return in_maps


def kernel(x, alpha, delta, beta, gamma, omega):
    global LAST_RESULTS
    if "nc" not in _CACHE:
        _CACHE["nc"] = _build_nc()
    nc = _CACHE["nc"]
    in_maps = _host_prep(x, alpha, delta, beta, gamma, omega)
    res = run_bass_kernel_spmd(nc, in_maps, core_ids=list(range(N_CORES)))
    LAST_RESULTS = res
    # untangle the partition-major store: [ss, p, (w b c)] -> [L, B, ESH]
    outs = []
    for c in range(N_CORES):
        o = res.results[c]["out"].reshape(NSS, 128, 2, B, ESH)
        outs.append(o.transpose(0, 2, 1, 3, 4).reshape(L, B, ESH))
    return np.concatenate(outs, axis=2).astype(np.float32)
